# revision 9
# baseline (speedup 1.0000x reference)
"""Trainium2 Bass kernel for nn_DeformableHistoryAttention_4148938408691.

Strategy (8 NeuronCores = 4 batches x 2 sequence halves, data parallel):
  Each core handles 2048 queries of one batch with a 1024-row K/V halo
  (3072 extended rows). All compute on device:
    - x transposed via PE (fp32 for the offset-MLP path, bf16 for Q/K)
    - offset MLP (W1/gelu/W2/tanh/mean) in fp32(r) -> deformable indices via
      exact round-half-even (2^23 trick), matching jnp.round bit-for-bit
      modulo fp accumulation-order noise
    - per-query index layout via 16 tiny PE transposes (no DRAM roundtrip)
    - dense windowed attention: per 128-query tile, scores over a static
      1152-wide causal window on the PE, evicted from PSUM through exp()
    - softmax-with-duplicates handled by multiplicity: an index-dedup pass
      produces per-query counts; gpsimd local_scatter writes the counts into
      the dense window; wdE = counts * exp(scores) gives unnormalized
      attention, Z = rowsum(wdE), and 1/Z is folded into the final output
      scaling (where partitions are queries)
    - AV uses associativity: attn @ (x@Wv) @ Wo == (attn @ x) @ (Wv@Wo);
      the fused weight Wf = Wv@Wo is precomputed on host, eliminating the
      V projection and the separate output projection
  Everything except the index path runs in bf16 with fp32 PSUM accumulation.
"""

import os
import sys

for _p in ("/opt/trn_rl_repo", "/root/.axon_site/_ro/trn_rl_repo"):
    if os.path.isdir(_p) and _p not in sys.path:
        sys.path.append(_p)

import dataclasses
from contextlib import ExitStack

import numpy as np

import concourse.bass as bass
import concourse.mybir as mybir
import concourse.tile as tile
from concourse import bacc
from concourse._compat import with_exitstack
from concourse.masks import make_identity
from concourse import library_config
from concourse.tile import add_dep_helper

F32 = mybir.dt.float32
F32R = mybir.dt.float32r
BF16 = mybir.dt.bfloat16
FP8 = mybir.dt.float8e4
I16 = mybir.dt.int16
FP8_SCORES = True      # Q/K in e4m3 for the dense score matmuls (2x PE rate)
AF = mybir.ActivationFunctionType
ALU = mybir.AluOpType

E = 512            # embed dim
H = 8              # heads
P = 16             # points
MAX_DIST = 1024
OFFSET_SCALE = 8.0
B, S = 4, 4096
NCORES = 8
SQ = 2048          # queries per core
EXT = 3072         # extended rows per core (1024 halo + 2048)
NT = 16            # query tiles of 128
W = 1152           # dense window width (1024 + 128)
EC = 4             # embed chunks of 128
RC = EXT // 128    # 24 row chunks
QRC = SQ // 128    # 16 query row chunks
RNE_C = float(2.0 ** 23)
N_GENERAL = 3      # tiles using general pairwise dedup (unsorted possible)


@with_exitstack
def _emit(ctx: ExitStack, tc: tile.TileContext, io: dict, reps: int = 1):
    nc = tc.nc

    const = ctx.enter_context(tc.tile_pool(name="const", bufs=1))

    # ---- identities & small constants ----
    ident_f = const.tile([128, 128], F32)
    make_identity(nc, ident_f)
    ident_b = const.tile([128, 128], BF16)
    make_identity(nc, ident_b)

    meanMf = const.tile([128, P], F32)
    nc.gpsimd.dma_start(meanMf[:], io["meanM"][:])
    meanM = const.tile([128, P], F32R)
    nc.vector.tensor_copy(meanM[:], meanMf[:])
    anchor = const.tile([P, SQ], F32)
    nc.gpsimd.dma_start(anchor[:], io["anchor"][:])
    clip_lo = const.tile([P, SQ], I16)
    nc.gpsimd.dma_start(clip_lo[:], io["clip_lo"][:])
    clip_hi = const.tile([P, SQ], I16)
    nc.gpsimd.dma_start(clip_hi[:], io["clip_hi"][:])
    tbase = const.tile([P, SQ], I16)
    nc.gpsimd.dma_start(tbase[:], io["tbase"][:])
    trimask = const.tile([128, P * P], F32)
    nc.gpsimd.dma_start(trimask[:], io["trimask"][:])

    # ---- weights ----
    # index-path weights as fp32r (walrus requires rounded producers)
    W1f = const.tile([128, EC, E], F32)
    nc.gpsimd.dma_start(W1f[:], io["W1"][:].rearrange("(kc p) m -> p kc m", p=128))
    W1s = const.tile([128, EC, E], F32R)
    nc.vector.tensor_copy(W1s[:], W1f[:])
    W2f = const.tile([128, EC, H * P], F32)
    nc.gpsimd.dma_start(W2f[:], io["W2"][:].rearrange("(kc p) m -> p kc m", p=128))
    W2s = const.tile([128, EC, H * P], F32R)
    nc.vector.tensor_copy(W2s[:], W2f[:])
    # bf16 weight tiles (DMAs emitted after phase 1 so x loads go first)
    Wqs = const.tile([128, EC, E], BF16)
    Wks = const.tile([128, EC, E], BF16)
    Wfs = const.tile([128, EC, E], BF16)   # fused Wv @ Wo

    for _rep in range(reps):
      with tc.tile_pool(name="persist", bufs=1) as persist:
        # ---- persistent activations ----
        xTb = persist.tile([128, EC, EXT], BF16)      # x^T bf16 (all ext rows)
        Xn = persist.tile([128, RC, E], BF16)         # x natural bf16 (for AV)
        qk_dt = FP8 if FP8_SCORES else BF16
        KT = persist.tile([128, EC, EXT], qk_dt)      # K^T
        QT = persist.tile([128, EC, SQ], qk_dt)       # Q^T
        idxS = persist.tile([128, NT, P], I16)        # per-query indices
        cnt = persist.tile([128, NT, P], F32)
        rep = persist.tile([128, NT, P], I16)
        cntv = persist.tile([128, NT, P], BF16)       # scatter values (counts)
        idxm = persist.tile([128, NT, P], I16)        # scatter indices (-1 = skip)
        rzt = persist.tile([128, NT], F32)            # 1/Z per query

        x_dram = io["x_ext"]
        out_dram = io["out"]

        # ================= Phase 1+2: x load, transpose, MLP -> indices ========
        with tc.tile_pool(name="ph12", bufs=2) as ph12, \
             tc.tile_pool(name="ph12p", bufs=2, space="PSUM") as ph12p:

            idx_f = ph12.tile([P, SQ], F32, bufs=1)

            # transpose all 24 row-chunks; fp32 x^T kept only for query rows,
            # streamed per 512-query group through the MLP
            xTf = None
            xpair = None
            for rc in range(RC):
                if rc % 2 == 0:
                    xpair = ph12.tile([128, 2, E], F32, tag="xch")
                    eng = nc.sync if (rc // 2) % 2 == 0 else nc.scalar
                    eng.dma_start(xpair[:], x_dram[:].rearrange("(r i p) e -> r p i e", p=128, i=2)[rc // 2])
                xch = xpair[:, rc % 2, :]
                nc.vector.tensor_copy(Xn[:, rc, :], xch)
                is_q = rc >= 8
                qrc = rc - 8            # query row chunk 0..15
                if is_q and qrc % 4 == 0:
                    xTf = ph12.tile([128, EC, 512], F32R, tag="xTf")
                for ec in range(EC):
                    tp = ph12p.tile([128, 128], F32, tag="tp")
                    nc.tensor.transpose(tp[:], xch[:, ec * 128:(ec + 1) * 128], ident_f[:])
                    nc.scalar.activation(xTb[:, ec, rc * 128:(rc + 1) * 128], tp[:], AF.Copy)
                    if is_q:
                        nc.vector.tensor_copy(xTf[:, ec, (qrc % 4) * 128:(qrc % 4 + 1) * 128], tp[:])
                if is_q and qrc % 4 == 3:
                    sc = qrc // 4       # 512-query group
                    ssl = slice(sc * 512, (sc + 1) * 512)
                    # MLP: h^T = gelu(W1^T x^T)
                    hT = ph12.tile([128, EC, 512], F32R, tag="hT", bufs=1)
                    for e1c in range(EC):
                        hp = ph12p.tile([128, 512], F32, tag="hp")
                        for kc in range(EC):
                            nc.tensor.matmul(hp[:], W1s[:, kc, e1c * 128:(e1c + 1) * 128],
                                             xTf[:, kc, :], start=kc == 0, stop=kc == EC - 1)
                        nc.scalar.activation(hT[:, e1c, :], hp[:], AF.Gelu)
                    # offsets: tanh(W2^T h^T)
                    op = ph12p.tile([128, 512], F32, tag="op")
                    for e1c in range(EC):
                        nc.tensor.matmul(op[:], W2s[:, e1c, :], hT[:, e1c, :],
                                         start=e1c == 0, stop=e1c == EC - 1)
                    tanhT = ph12.tile([128, 512], F32R, tag="tanhT")
                    nc.scalar.activation(tanhT[:], op[:], AF.Tanh)
                    # mean over heads: [16, 512]
                    mp = ph12p.tile([P, 512], F32, tag="mp")
                    nc.tensor.matmul(mp[:], meanM[:], tanhT[:], start=True, stop=True)
                    # sampled = clip(anchor + 8*mean, lo, hi); idx = rne(sampled)
                    sf = ph12.tile([P, 512], F32, tag="sf")
                    nc.vector.scalar_tensor_tensor(sf[:], mp[:], float(OFFSET_SCALE),
                                                   anchor[:, ssl], op0=ALU.mult, op1=ALU.add)
                    nc.vector.tensor_tensor(sf[:], sf[:], clip_lo[:, ssl], op=ALU.max)
                    nc.vector.tensor_tensor(sf[:], sf[:], clip_hi[:, ssl], op=ALU.min)
                    nc.vector.tensor_scalar_add(sf[:], sf[:], RNE_C)
                    nc.vector.tensor_scalar_add(sf[:], sf[:], -RNE_C)
                    nc.vector.tensor_tensor(idx_f[:, ssl], sf[:], tbase[:, ssl], op=ALU.subtract)

            # ---- per-query index layout via tiny PE transposes ----
            idxS_f = ph12.tile([128, NT, P], F32, bufs=1, tag="idxSf")
            for t in range(NT):
                tpi = ph12p.tile([128, 128], F32, tag="tp")
                nc.tensor.transpose(tpi[:, 0:P], idx_f[:, t * 128:(t + 1) * 128],
                                    ident_f[0:P, 0:P])
                nc.vector.tensor_copy(idxS_f[:, t, :], tpi[:, 0:P])
            nc.vector.tensor_copy(idxS[:], idxS_f[:])

        # bf16 weights (cast during DMA via SWDGE)
        nc.gpsimd.dma_start(Wqs[:], io["Wq"][:].rearrange("(kc p) m -> p kc m", p=128))
        nc.gpsimd.dma_start(Wks[:], io["Wk"][:].rearrange("(kc p) m -> p kc m", p=128))
        nc.gpsimd.dma_start(Wfs[:], io["Wf"][:].rearrange("(kc p) m -> p kc m", p=128))

        # ---- dedup: cnt = multiplicity, rep = first-occurrence mask ----
        nc.vector.memset(cnt[:], 1.0)
        eqt = persist.tile([128, NT, P], F32)
        for L in range(1, P):
            nc.vector.tensor_tensor(eqt[:, :, :P - L], idxS[:, :, L:], idxS[:, :, :P - L],
                                    op=ALU.is_equal)
            nc.vector.tensor_tensor(cnt[:, :, :P - L], cnt[:, :, :P - L], eqt[:, :, :P - L],
                                    op=ALU.add)
        nc.vector.memset(rep[:, :, 0:1], 1.0)
        nc.vector.tensor_tensor(rep[:, :, 1:], idxS[:, :, 1:], idxS[:, :, :P - 1],
                                op=ALU.not_equal)
        # general pairwise for the first N_GENERAL tiles (may be unsorted)
        eqm = persist.tile([128, N_GENERAL, P, P], F32)
        in0 = idxS[:, :N_GENERAL, :].to_broadcast([128, N_GENERAL, P, P])
        in1 = in0.rearrange("c t p b -> c t b p")
        nc.vector.tensor_tensor(eqm[:], in0, in1, op=ALU.is_equal)
        nc.vector.reduce_sum(cnt[:, :N_GENERAL, :], eqm[:], axis=mybir.AxisListType.X)
        tri = trimask[:].rearrange("c (p b) -> c p b", p=P)
        tri = dataclasses.replace(
            tri, ap=[tri.ap[0], [0, N_GENERAL], tri.ap[1], tri.ap[2]])
        nc.vector.tensor_tensor(eqm[:], eqm[:], tri, op=ALU.mult)
        nbef = persist.tile([128, N_GENERAL, P], F32)
        nc.vector.reduce_sum(nbef[:], eqm[:], axis=mybir.AxisListType.X)
        nc.vector.tensor_scalar(rep[:, :N_GENERAL, :], nbef[:], 0.0, None, op0=ALU.is_equal)

        nc.vector.tensor_copy(cntv[:], cnt[:])
        nc.vector.memset(idxm[:], -1)
        nc.vector.copy_predicated(idxm[:], rep[:], idxS[:])

        # ================= Phase 4: projections (Q^T, K^T) ======================
        with tc.tile_pool(name="psum_s", bufs=2, space="PSUM") as psum_s:
            for mc in range(EC):        # K^T / Q^T output embed chunk
                for nc_i in range(RC // 4):   # 512-col groups of ext rows
                    ksl = slice(nc_i * 512, (nc_i + 1) * 512)
                    kp = psum_s.tile([128, 512], F32, tag="projp")
                    for kc in range(EC):
                        nc.tensor.matmul(kp[:], Wks[:, kc, mc * 128:(mc + 1) * 128],
                                         xTb[:, kc, ksl], start=kc == 0, stop=kc == EC - 1)
                    nc.scalar.activation(KT[:, mc, ksl], kp[:], AF.Copy)
                for nc_i in range(QRC // 4):
                    qsl = slice(1024 + nc_i * 512, 1024 + (nc_i + 1) * 512)
                    qp = psum_s.tile([128, 512], F32, tag="projp")
                    for kc in range(EC):
                        nc.tensor.matmul(qp[:], Wqs[:, kc, mc * 128:(mc + 1) * 128],
                                         xTb[:, kc, qsl], start=kc == 0, stop=kc == EC - 1)
                    nc.scalar.activation(QT[:, mc, slice(nc_i * 512, (nc_i + 1) * 512)], qp[:], AF.Copy)

        # ================= Phase 5: scores+exp, scatter, weights, AV ============
        lib7 = nc.gpsimd.load_library(library_config.local_scatter)
        NCHUNKS = ((0, 512), (512, 512), (1024, 128))
        NP_PAIR = NT // 2
        with tc.tile_pool(name="ph5", bufs=2) as ph5, \
             tc.tile_pool(name="ph5s", bufs=2, space="PSUM") as ph5s, \
             tc.tile_pool(name="ph5t", bufs=2, space="PSUM") as ph5t, \
             tc.tile_pool(name="ph5a", bufs=1, space="PSUM") as ph5a, \
             tc.tile_pool(name="ph5o", bufs=2, space="PSUM") as ph5o:
            for pr in range(NP_PAIR):
                wT = ph5.tile([128, 10, 256], BF16, tag="wT")
                nc.vector.memset(wT[:, 9, 0:128], 0.0)
                nc.vector.memset(wT[:, 0, 128:256], 0.0)
                for wh in range(2):
                    t = pr * 2 + wh
                    # dense scores, evicted from PSUM through exp()
                    Et = ph5.tile([128, W], BF16, tag="Et")
                    for noff, nw in NCHUNKS:
                        sp = ph5s.tile([128, 512], F32, tag="sp")
                        for ec in range(EC):
                            nc.tensor.matmul(sp[:, 0:nw],
                                             QT[:, ec, t * 128:(t + 1) * 128],
                                             KT[:, ec, t * 128 + noff:t * 128 + noff + nw],
                                             start=ec == 0, stop=ec == EC - 1)
                        nc.scalar.activation(Et[:, noff:noff + nw], sp[:, 0:nw], AF.Exp,
                                             scale=float(1.0 / np.sqrt(E)))
                    # multiplicity counts scattered into the dense window
                    wd = ph5.tile([128, W], BF16, tag="wd")
                    si = nc.gpsimd.local_scatter(wd[:], cntv[:, t, :], idxm[:, t, :],
                                                 channels=128, num_elems=W, num_idxs=P)
                    add_dep_helper(si.ins, lib7.ins, False, "lib7 before scatters")
                    # unnormalized attention + row sum (mult on gpsimd: same
                    # queue as the scatter, keeps DVE off the PE critical path)
                    wdE = ph5.tile([128, W], BF16, tag="wdE")
                    nc.gpsimd.tensor_tensor(wdE[:], wd[:], Et[:], op=ALU.mult)
                    zs = ph5.tile([128, 1], F32, tag="zs")
                    nc.vector.reduce_sum(zs[:], wdE[:], axis=mybir.AxisListType.X)
                    nc.vector.reciprocal(rzt[:, t:t + 1], zs[:])
                    for jc in range(9):
                        tpb = ph5t.tile([128, 128], BF16, tag="tpb")
                        nc.tensor.transpose(tpb[:], wdE[:, jc * 128:(jc + 1) * 128], ident_b[:])
                        nc.vector.tensor_copy(wT[:, jc + wh, wh * 128:(wh + 1) * 128], tpb[:])
                avp = ph5a.tile([128, EC * 256], F32, tag="avp")
                for ec in range(EC):
                    for jc in range(10):
                        nc.tensor.matmul(avp[:, ec * 256:(ec + 1) * 256],
                                         Xn[:, pr * 2 + jc, ec * 128:(ec + 1) * 128],
                                         wT[:, jc, :], start=jc == 0, stop=jc == 9)
                avT = ph5.tile([128, EC, 256], BF16, tag="avT")
                nc.vector.tensor_copy(avT[:], avp[:].rearrange("c (e s) -> c e s", e=EC))
                for wh in range(2):
                    t = pr * 2 + wh
                    wop = ph5o.tile([128, E], F32, tag="wop")
                    for ec in range(EC):
                        nc.tensor.matmul(wop[:], avT[:, ec, wh * 128:(wh + 1) * 128],
                                         Wfs[:, ec, :], start=ec == 0, stop=ec == EC - 1)
                    osb = ph5.tile([128, E], F32, tag="osb")
                    nc.scalar.activation(osb[:], wop[:], AF.Copy,
                                         scale=rzt[:, t:t + 1])
                    nc.sync.dma_start(
                        out_dram[:].rearrange("(t p) e -> t p e", p=128)[t], osb[:])


def build_nc(reps: int = 1):
    nc = bacc.Bacc("TRN2", target_bir_lowering=False, debug=False)
    io = {}
    io["x_ext"] = nc.declare_dram_parameter("x_ext", [EXT, E], F32, isOutput=False).ap()
    for nm in ("Wq", "Wk", "Wf", "W1", "W2"):
        shp = [E, H * P] if nm == "W2" else [E, E]
        io[nm] = nc.declare_dram_parameter(nm, shp, F32, isOutput=False).ap()
    io["anchor"] = nc.declare_dram_parameter("anchor", [P, SQ], F32, isOutput=False).ap()
    io["clip_lo"] = nc.declare_dram_parameter("clip_lo", [P, SQ], I16, isOutput=False).ap()
    io["clip_hi"] = nc.declare_dram_parameter("clip_hi", [P, SQ], I16, isOutput=False).ap()
    io["tbase"] = nc.declare_dram_parameter("tbase", [P, SQ], I16, isOutput=False).ap()
    io["meanM"] = nc.declare_dram_parameter("meanM", [128, P], F32, isOutput=False).ap()
    io["trimask"] = nc.declare_dram_parameter("trimask", [128, P * P], F32, isOutput=False).ap()
    io["out"] = nc.declare_dram_parameter("out", [SQ, E], F32, isOutput=True).ap()

    with tile.TileContext(nc) as tc:
        _emit(tc, io, reps=reps)
    nc.finalize()
    return nc


def host_inputs(inputs: dict) -> list:
    """Build the 8 per-core input maps from the full problem inputs."""
    x = np.asarray(inputs["x"], np.float32)
    anchors = np.asarray(inputs["anchors"], np.float32)
    weights = {k: np.ascontiguousarray(np.asarray(inputs[k], np.float32))
               for k in ("Wq", "Wk", "W1", "W2")}
    weights["Wf"] = np.ascontiguousarray(
        np.asarray(inputs["Wv"], np.float32) @ np.asarray(inputs["Wo"], np.float32))

    meanM = np.zeros((128, P), np.float32)
    for hp in range(128):
        meanM[hp, hp % P] = 1.0 / H
    tri = np.tile(np.tril(np.ones((P, P), np.float32), -1).reshape(1, P * P), (128, 1))
    tbase = np.tile((np.arange(SQ, dtype=np.int64) // 128 * 128)[None, :], (P, 1)).astype(np.int16)

    in_maps = []
    for c in range(NCORES):
        b, h = c // 2, c % 2
        if h == 0:
            x_ext = np.concatenate([np.zeros((1024, E), np.float32), x[b, :2048]], 0)
        else:
            x_ext = np.ascontiguousarray(x[b, 1024:4096])
        shift = np.float32(1024 - h * 2048)
        s_abs = np.arange(h * 2048, h * 2048 + SQ, dtype=np.float32)
        anchor_term = anchors[:, None] * s_abs[None, :] + shift          # [16, 2048]
        lo = (np.maximum(s_abs - MAX_DIST, 0.0) + shift).astype(np.int16)
        hi = (s_abs + shift).astype(np.int16)
        m = {
            "partition_id": np.array([[c]], np.uint32),
            "x_ext": x_ext,
            "anchor": anchor_term.astype(np.float32),
            "clip_lo": np.tile(lo[None, :], (P, 1)),
            "clip_hi": np.tile(hi[None, :], (P, 1)),
            "tbase": tbase,
            "meanM": meanM,
            "trimask": tri,
        }
        m.update(weights)
        in_maps.append(m)
    return in_maps


_CACHE = {}


def get_runner():
    """Build (once) a cached jitted SPMD callable over the 8 cores.

    Returns (run, in_names) where run takes a list of per-input np arrays
    concatenated over cores on axis 0 and returns the concatenated outputs.
    """
    if "run" in _CACHE:
        return _CACHE["run"], _CACHE["in_names"]

    import jax
    from jax.experimental.shard_map import shard_map
    from jax.sharding import Mesh, PartitionSpec
    import concourse.mybir as _mb
    from concourse.bass2jax import _bass_exec_p, install_neuronx_cc_hook

    nc = build_nc()
    install_neuronx_cc_hook()

    in_names, out_names, out_avals, zero_outs = [], [], [], []
    for alloc in nc.m.functions[0].allocations:
        if not isinstance(alloc, _mb.MemoryLocationSet):
            continue
        name = alloc.memorylocations[0].name
        if alloc.kind == "ExternalInput":
            in_names.append(name)
        elif alloc.kind == "ExternalOutput":
            out_names.append(name)
            shape = tuple(alloc.tensor_shape)
            dtype = _mb.dt.np(alloc.dtype)
            out_avals.append(jax.core.ShapedArray(shape, dtype))
            zero_outs.append(np.zeros((NCORES * shape[0], *shape[1:]), dtype))

    n_params = len(in_names)
    all_names = in_names + out_names

    def _body(*args):
        outs = _bass_exec_p.bind(
            *args,
            out_avals=tuple(out_avals),
            in_names=tuple(all_names),
            out_names=tuple(out_names),
            lowering_input_output_aliases=(),
            sim_require_finite=True,
            sim_require_nnan=True,
            nc=nc,
        )
        return tuple(outs)

    devices = jax.devices()[:NCORES]
    mesh = Mesh(np.asarray(devices), ("core",))
    sharded = jax.jit(
        shard_map(_body, mesh=mesh,
                  in_specs=(PartitionSpec("core"),) * (n_params + len(out_names)),
                  out_specs=(PartitionSpec("core"),) * len(out_names),
                  check_rep=False),
        keep_unused=True,
    )

    def run(concat_ins):
        outs = sharded(*concat_ins, *zero_outs)
        return [np.asarray(o) for o in outs]

    _CACHE.update(run=run, in_names=in_names, sharded=sharded, zero_outs=zero_outs)
    return run, in_names


def concat_inputs(in_maps, in_names):
    return [np.concatenate([np.asarray(m[n]) for m in in_maps], axis=0)
            for n in in_names]


def kernel(**inputs) -> np.ndarray:
    run, in_names = get_runner()
    in_maps = host_inputs(inputs)
    res = run(concat_inputs(in_maps, in_names))[0]   # [NCORES*SQ, E]
    out = np.zeros((B, S, E), np.float32)
    for c in range(NCORES):
        b, h = c // 2, c % 2
        out[b, h * 2048:(h + 1) * 2048] = res[c * SQ:(c + 1) * SQ]
    return out


# revision 15
# speedup vs baseline: 1.4875x; 1.4875x over previous
"""Trainium2 Bass kernel for nn_DeformableHistoryAttention_4148938408691.

Strategy (8 NeuronCores = 4 batches x 2 sequence halves, data parallel):
  Each core handles 2048 queries of one batch with a 1024-row K/V halo
  (3072 extended rows). All compute on device:
    - x transposed via PE (fp32 for the offset-MLP path, bf16 for Q/K)
    - offset MLP (W1/gelu/W2/tanh/mean) in fp32(r) -> deformable indices via
      exact round-half-even (2^23 trick), matching jnp.round bit-for-bit
      modulo fp accumulation-order noise
    - per-query index layout via 16 tiny PE transposes (no DRAM roundtrip)
    - dense windowed attention: per 128-query tile, scores over a static
      1152-wide causal window on the PE, evicted from PSUM through exp()
    - softmax-with-duplicates handled by multiplicity: an index-dedup pass
      produces per-query counts; gpsimd local_scatter writes the counts into
      the dense window; wdE = counts * exp(scores) gives unnormalized
      attention, Z = rowsum(wdE), and 1/Z is folded into the final output
      scaling (where partitions are queries)
    - AV uses associativity: attn @ (x@Wv) @ Wo == (attn @ x) @ (Wv@Wo);
      the fused weight Wf = Wv@Wo is precomputed on host, eliminating the
      V projection and the separate output projection
  Everything except the index path runs in bf16 with fp32 PSUM accumulation.
"""

import os
import sys

for _p in ("/opt/trn_rl_repo", "/root/.axon_site/_ro/trn_rl_repo"):
    if os.path.isdir(_p) and _p not in sys.path:
        sys.path.append(_p)

import dataclasses
from contextlib import ExitStack

import numpy as np

import concourse.bass as bass
import concourse.mybir as mybir
import concourse.tile as tile
from concourse import bacc
from concourse._compat import with_exitstack
from concourse.masks import make_identity
from concourse import library_config
from concourse.tile import add_dep_helper

F32 = mybir.dt.float32
F32R = mybir.dt.float32r
BF16 = mybir.dt.bfloat16
FP8 = mybir.dt.float8e4
I16 = mybir.dt.int16
FP8_SCORES = True      # Q/K in e4m3 for the dense score matmuls (2x PE rate)
AF = mybir.ActivationFunctionType
ALU = mybir.AluOpType

E = 512            # embed dim
H = 8              # heads
P = 16             # points
MAX_DIST = 1024
OFFSET_SCALE = 8.0
B, S = 4, 4096
NCORES = 8
SQ = 2048          # queries per core
EXT = 3072         # extended rows per core (1024 halo + 2048)
NT = 16            # query tiles of 128
W = 1152           # dense window width (1024 + 128)
EC = 4             # embed chunks of 128
RC = EXT // 128    # 24 row chunks
QRC = SQ // 128    # 16 query row chunks
RNE_C = float(2.0 ** 23)
N_GENERAL = 3      # tiles using general pairwise dedup (unsorted possible)


@with_exitstack
def _emit(ctx: ExitStack, tc: tile.TileContext, io: dict, reps: int = 1):
    nc = tc.nc

    const = ctx.enter_context(tc.tile_pool(name="const", bufs=1))

    # ---- identities & small constants ----
    ident_f = const.tile([128, 128], F32)
    make_identity(nc, ident_f)
    ident_b = const.tile([128, 128], BF16)
    make_identity(nc, ident_b)

    meanMf = const.tile([128, P], F32)
    nc.gpsimd.dma_start(meanMf[:], io["meanM"][:])
    meanM = const.tile([128, P], F32R)
    nc.vector.tensor_copy(meanM[:], meanMf[:])
    anchor = const.tile([P, SQ], F32)
    nc.gpsimd.dma_start(anchor[:], io["anchor"][:])
    clip_lo = const.tile([P, SQ], I16)
    nc.gpsimd.dma_start(clip_lo[:], io["clip_lo"][:])
    clip_hi = const.tile([P, SQ], I16)
    nc.gpsimd.dma_start(clip_hi[:], io["clip_hi"][:])
    tbase = const.tile([P, SQ], I16)
    nc.gpsimd.dma_start(tbase[:], io["tbase"][:])
    trimask = const.tile([128, P * P], F32)
    nc.gpsimd.dma_start(trimask[:], io["trimask"][:])

    # ---- weights ----
    # index-path weights as fp32r (walrus requires rounded producers)
    W1f = const.tile([128, EC, E], F32)
    nc.gpsimd.dma_start(W1f[:], io["W1"][:].rearrange("(kc p) m -> p kc m", p=128))
    W1s = const.tile([128, EC, E], F32R)
    nc.vector.tensor_copy(W1s[:], W1f[:])
    W2f = const.tile([128, EC, H * P], F32)
    nc.gpsimd.dma_start(W2f[:], io["W2"][:].rearrange("(kc p) m -> p kc m", p=128))
    W2s = const.tile([128, EC, H * P], F32R)
    nc.vector.tensor_copy(W2s[:], W2f[:])
    # bf16 weight tiles (DMAs emitted after phase 1 so x loads go first)
    Wqs = const.tile([128, EC, E], BF16)
    Wks = const.tile([128, EC, E], BF16)
    Wfs = const.tile([128, EC, E], BF16)   # fused Wv @ Wo

    for _rep in range(reps):
      with tc.tile_pool(name="persist", bufs=1) as persist:
        # ---- persistent activations ----
        xTb = persist.tile([128, EC, EXT], BF16)      # x^T bf16 (all ext rows)
        Xn = persist.tile([128, RC, E], BF16)         # x natural bf16 (for AV)
        qk_dt = FP8 if FP8_SCORES else BF16
        KT = persist.tile([128, EC, EXT], qk_dt)      # K^T
        QT = persist.tile([128, EC, SQ], qk_dt)       # Q^T
        idxS = persist.tile([128, NT, P], I16)        # per-query indices
        cnt = persist.tile([128, NT, P], F32)
        rep = persist.tile([128, NT, P], I16)
        cntv = persist.tile([128, NT, P], BF16)       # scatter values (counts)
        idxm = persist.tile([128, NT, P], I16)        # scatter indices (-1 = skip)
        rzt = persist.tile([128, NT], F32)            # 1/Z per query

        x_dram = io["x_ext"]
        out_dram = io["out"]

        # ================= Phase 1+2: x load, transpose, MLP -> indices ========
        with tc.tile_pool(name="ph12", bufs=2) as ph12, \
             tc.tile_pool(name="ph12p", bufs=2, space="PSUM") as ph12p:

            idx_f = ph12.tile([P, SQ], F32, bufs=1)

            # transpose all 24 row-chunks; fp32 x^T kept only for query rows,
            # streamed per 512-query group through the MLP
            xTf = None
            x_rows = x_dram[:].rearrange("(r p) e -> r p e", p=128)
            for rc in range(RC):
                xcht = ph12.tile([128, E], F32, tag="xch", bufs=6)
                eng = (nc.sync, nc.scalar)[rc % 2]
                eng.dma_start(xcht[:], x_rows[rc])
                xch = xcht[:]
                nc.vector.tensor_copy(Xn[:, rc, :], xch)
                is_q = rc >= 8
                qrc = rc - 8            # query row chunk 0..15
                if is_q and qrc % 4 == 0:
                    xTf = ph12.tile([128, EC, 512], F32R, tag="xTf")
                for ec in range(EC):
                    tp = ph12p.tile([128, 128], F32, tag="tp")
                    nc.tensor.transpose(tp[:], xch[:, ec * 128:(ec + 1) * 128], ident_f[:])
                    nc.scalar.activation(xTb[:, ec, rc * 128:(rc + 1) * 128], tp[:], AF.Copy)
                    if is_q:
                        nc.vector.tensor_copy(xTf[:, ec, (qrc % 4) * 128:(qrc % 4 + 1) * 128], tp[:])
                if is_q and qrc % 4 == 3:
                    sc = qrc // 4       # 512-query group
                    ssl = slice(sc * 512, (sc + 1) * 512)
                    # MLP: h^T = gelu(W1^T x^T)
                    hT = ph12.tile([128, EC, 512], F32R, tag="hT", bufs=1)
                    for e1c in range(EC):
                        hp = ph12p.tile([128, 512], F32, tag="hp")
                        for kc in range(EC):
                            nc.tensor.matmul(hp[:], W1s[:, kc, e1c * 128:(e1c + 1) * 128],
                                             xTf[:, kc, :], start=kc == 0, stop=kc == EC - 1)
                        nc.scalar.activation(hT[:, e1c, :], hp[:], AF.Gelu)
                    # offsets: tanh(W2^T h^T)
                    op = ph12p.tile([128, 512], F32, tag="op")
                    for e1c in range(EC):
                        nc.tensor.matmul(op[:], W2s[:, e1c, :], hT[:, e1c, :],
                                         start=e1c == 0, stop=e1c == EC - 1)
                    tanhT = ph12.tile([128, 512], F32R, tag="tanhT")
                    nc.scalar.activation(tanhT[:], op[:], AF.Tanh)
                    # mean over heads: [16, 512]
                    mp = ph12p.tile([P, 512], F32, tag="mp")
                    nc.tensor.matmul(mp[:], meanM[:], tanhT[:], start=True, stop=True)
                    # sampled = clip(anchor + 8*mean, lo, hi); idx = rne(sampled)
                    sf = ph12.tile([P, 512], F32, tag="sf")
                    nc.vector.scalar_tensor_tensor(sf[:], mp[:], float(OFFSET_SCALE),
                                                   anchor[:, ssl], op0=ALU.mult, op1=ALU.add)
                    nc.vector.tensor_tensor(sf[:], sf[:], clip_lo[:, ssl], op=ALU.max)
                    nc.vector.tensor_tensor(sf[:], sf[:], clip_hi[:, ssl], op=ALU.min)
                    nc.vector.tensor_scalar_add(sf[:], sf[:], RNE_C)
                    nc.vector.tensor_scalar_add(sf[:], sf[:], -RNE_C)
                    nc.vector.tensor_tensor(idx_f[:, ssl], sf[:], tbase[:, ssl], op=ALU.subtract)

            # ---- per-query index layout via tiny PE transposes ----
            idxS_f = ph12.tile([128, NT, P], F32, bufs=1, tag="idxSf")
            for t in range(NT):
                tpi = ph12p.tile([128, 128], F32, tag="tp")
                nc.tensor.transpose(tpi[:, 0:P], idx_f[:, t * 128:(t + 1) * 128],
                                    ident_f[0:P, 0:P])
                nc.vector.tensor_copy(idxS_f[:, t, :], tpi[:, 0:P])
            nc.vector.tensor_copy(idxS[:], idxS_f[:])

        # bf16 weights (cast during DMA via SWDGE)
        nc.gpsimd.dma_start(Wqs[:], io["Wq"][:].rearrange("(kc p) m -> p kc m", p=128))
        nc.gpsimd.dma_start(Wks[:], io["Wk"][:].rearrange("(kc p) m -> p kc m", p=128))
        nc.gpsimd.dma_start(Wfs[:], io["Wf"][:].rearrange("(kc p) m -> p kc m", p=128))

        # ---- dedup: cnt = multiplicity, rep = first-occurrence mask ----
        nc.vector.memset(cnt[:], 1.0)
        eqt = persist.tile([128, NT, P], F32)
        for L in range(1, P):
            nc.vector.tensor_tensor(eqt[:, :, :P - L], idxS[:, :, L:], idxS[:, :, :P - L],
                                    op=ALU.is_equal)
            nc.vector.tensor_tensor(cnt[:, :, :P - L], cnt[:, :, :P - L], eqt[:, :, :P - L],
                                    op=ALU.add)
        nc.vector.memset(rep[:, :, 0:1], 1.0)
        nc.vector.tensor_tensor(rep[:, :, 1:], idxS[:, :, 1:], idxS[:, :, :P - 1],
                                op=ALU.not_equal)
        # general pairwise for the first N_GENERAL tiles (may be unsorted)
        eqm = persist.tile([128, N_GENERAL, P, P], F32)
        in0 = idxS[:, :N_GENERAL, :].to_broadcast([128, N_GENERAL, P, P])
        in1 = in0.rearrange("c t p b -> c t b p")
        nc.vector.tensor_tensor(eqm[:], in0, in1, op=ALU.is_equal)
        nc.vector.reduce_sum(cnt[:, :N_GENERAL, :], eqm[:], axis=mybir.AxisListType.X)
        tri = trimask[:].rearrange("c (p b) -> c p b", p=P)
        tri = dataclasses.replace(
            tri, ap=[tri.ap[0], [0, N_GENERAL], tri.ap[1], tri.ap[2]])
        nc.vector.tensor_tensor(eqm[:], eqm[:], tri, op=ALU.mult)
        nbef = persist.tile([128, N_GENERAL, P], F32)
        nc.vector.reduce_sum(nbef[:], eqm[:], axis=mybir.AxisListType.X)
        nc.vector.tensor_scalar(rep[:, :N_GENERAL, :], nbef[:], 0.0, None, op0=ALU.is_equal)

        nc.vector.tensor_copy(cntv[:], cnt[:])
        nc.vector.memset(idxm[:], -1)
        nc.vector.copy_predicated(idxm[:], rep[:], idxS[:])

        # ================= Phase 4: projections (Q^T, K^T) ======================
        with tc.tile_pool(name="psum_s", bufs=2, space="PSUM") as psum_s:
            for mc in range(EC):        # K^T / Q^T output embed chunk
                for nc_i in range(RC // 4):   # 512-col groups of ext rows
                    ksl = slice(nc_i * 512, (nc_i + 1) * 512)
                    kp = psum_s.tile([128, 512], F32, tag="projp")
                    for kc in range(EC):
                        nc.tensor.matmul(kp[:], Wks[:, kc, mc * 128:(mc + 1) * 128],
                                         xTb[:, kc, ksl], start=kc == 0, stop=kc == EC - 1)
                    nc.scalar.activation(KT[:, mc, ksl], kp[:], AF.Copy)
                for nc_i in range(QRC // 4):
                    qsl = slice(1024 + nc_i * 512, 1024 + (nc_i + 1) * 512)
                    qp = psum_s.tile([128, 512], F32, tag="projp")
                    for kc in range(EC):
                        nc.tensor.matmul(qp[:], Wqs[:, kc, mc * 128:(mc + 1) * 128],
                                         xTb[:, kc, qsl], start=kc == 0, stop=kc == EC - 1)
                    nc.scalar.activation(QT[:, mc, slice(nc_i * 512, (nc_i + 1) * 512)], qp[:], AF.Copy)

        # ================= Phase 5: scores+exp, scatter, weights, AV ============
        lib7 = nc.gpsimd.load_library(library_config.local_scatter)
        NCHUNKS = ((0, 512), (512, 512), (1024, 128))
        NP_PAIR = NT // 2
        with tc.tile_pool(name="ph5", bufs=2) as ph5, \
             tc.tile_pool(name="ph5s", bufs=2, space="PSUM") as ph5s, \
             tc.tile_pool(name="ph5t", bufs=2, space="PSUM") as ph5t, \
             tc.tile_pool(name="ph5a", bufs=1, space="PSUM") as ph5a, \
             tc.tile_pool(name="ph5o", bufs=2, space="PSUM") as ph5o:
            for pr in range(NP_PAIR):
                wT = ph5.tile([128, 10, 256], BF16, tag="wT")
                nc.vector.memset(wT[:, 9, 0:128], 0.0)
                nc.vector.memset(wT[:, 0, 128:256], 0.0)
                wdE_pair = []
                for wh in range(2):
                    t = pr * 2 + wh
                    # dense scores, evicted from PSUM through exp()
                    Et = ph5.tile([128, W], BF16, tag="Et")
                    for noff, nw in NCHUNKS:
                        sp = ph5s.tile([128, 512], F32, tag="sp")
                        tsl = slice(t * 128, (t + 1) * 128)
                        wsl = slice(t * 128 + noff, t * 128 + noff + nw)
                        if FP8_SCORES:
                            # DoubleRow: two ec-chunk contractions per pass
                            for pe in (0, 2):
                                nc.tensor.matmul(
                                    sp[:, 0:nw], QT[:, pe:pe + 2, tsl],
                                    KT[:, pe:pe + 2, wsl],
                                    start=pe == 0, stop=pe == 2,
                                    perf_mode=mybir.MatmulPerfMode.DoubleRow)
                        else:
                            for ec in range(EC):
                                nc.tensor.matmul(sp[:, 0:nw],
                                                 QT[:, ec, tsl], KT[:, ec, wsl],
                                                 start=ec == 0, stop=ec == EC - 1)
                        nc.scalar.activation(Et[:, noff:noff + nw], sp[:, 0:nw], AF.Exp,
                                             scale=float(1.0 / np.sqrt(E)))
                    # multiplicity counts scattered into the dense window
                    wd = ph5.tile([128, W], BF16, tag="wd")
                    si = nc.gpsimd.local_scatter(wd[:], cntv[:, t, :], idxm[:, t, :],
                                                 channels=128, num_elems=W, num_idxs=P)
                    add_dep_helper(si.ins, lib7.ins, False, "lib7 before scatters")
                    # unnormalized attention + row sum
                    wdE = ph5.tile([128, W], BF16, tag="wdE")
                    nc.vector.tensor_tensor(wdE[:], wd[:], Et[:], op=ALU.mult)
                    zs = ph5.tile([128, 1], F32, tag="zs")
                    nc.vector.reduce_sum(zs[:], wdE[:], axis=mybir.AxisListType.X)
                    nc.vector.reciprocal(rzt[:, t:t + 1], zs[:])
                    wdE_pair.append(wdE)
                # transposes after both tiles' scores: while the PE works on
                # tile t1's scores, t0's scatter/mult complete off-engine
                for wh in range(2):
                    wdE = wdE_pair[wh]
                    for jc in range(9):
                        tpb = ph5t.tile([128, 128], BF16, tag="tpb")
                        nc.tensor.transpose(tpb[:], wdE[:, jc * 128:(jc + 1) * 128], ident_b[:])
                        nc.vector.tensor_copy(wT[:, jc + wh, wh * 128:(wh + 1) * 128], tpb[:])
                avp = ph5a.tile([128, EC * 256], F32, tag="avp")
                for ec in range(EC):
                    for jc in range(10):
                        nc.tensor.matmul(avp[:, ec * 256:(ec + 1) * 256],
                                         Xn[:, pr * 2 + jc, ec * 128:(ec + 1) * 128],
                                         wT[:, jc, :], start=jc == 0, stop=jc == 9)
                avT = ph5.tile([128, EC, 256], BF16, tag="avT")
                nc.vector.tensor_copy(avT[:], avp[:].rearrange("c (e s) -> c e s", e=EC))
                for wh in range(2):
                    t = pr * 2 + wh
                    wop = ph5o.tile([128, E], F32, tag="wop")
                    for ec in range(EC):
                        nc.tensor.matmul(wop[:], avT[:, ec, wh * 128:(wh + 1) * 128],
                                         Wfs[:, ec, :], start=ec == 0, stop=ec == EC - 1)
                    osb = ph5.tile([128, E], F32, tag="osb")
                    nc.scalar.activation(osb[:], wop[:], AF.Copy,
                                         scale=rzt[:, t:t + 1])
                    nc.sync.dma_start(
                        out_dram[:].rearrange("(t p) e -> t p e", p=128)[t], osb[:])


def build_nc(reps: int = 1):
    nc = bacc.Bacc("TRN2", target_bir_lowering=False, debug=False)
    io = {}
    io["x_ext"] = nc.declare_dram_parameter("x_ext", [EXT, E], F32, isOutput=False).ap()
    for nm in ("Wq", "Wk", "Wf", "W1", "W2"):
        shp = [E, H * P] if nm == "W2" else [E, E]
        io[nm] = nc.declare_dram_parameter(nm, shp, F32, isOutput=False).ap()
    io["anchor"] = nc.declare_dram_parameter("anchor", [P, SQ], F32, isOutput=False).ap()
    io["clip_lo"] = nc.declare_dram_parameter("clip_lo", [P, SQ], I16, isOutput=False).ap()
    io["clip_hi"] = nc.declare_dram_parameter("clip_hi", [P, SQ], I16, isOutput=False).ap()
    io["tbase"] = nc.declare_dram_parameter("tbase", [P, SQ], I16, isOutput=False).ap()
    io["meanM"] = nc.declare_dram_parameter("meanM", [128, P], F32, isOutput=False).ap()
    io["trimask"] = nc.declare_dram_parameter("trimask", [128, P * P], F32, isOutput=False).ap()
    io["out"] = nc.declare_dram_parameter("out", [SQ, E], F32, isOutput=True).ap()

    with tile.TileContext(nc) as tc:
        _emit(tc, io, reps=reps)
    nc.finalize()
    return nc


def host_inputs(inputs: dict) -> list:
    """Build the 8 per-core input maps from the full problem inputs."""
    x = np.asarray(inputs["x"], np.float32)
    anchors = np.asarray(inputs["anchors"], np.float32)
    weights = {k: np.ascontiguousarray(np.asarray(inputs[k], np.float32))
               for k in ("Wq", "Wk", "W1", "W2")}
    weights["Wf"] = np.ascontiguousarray(
        np.asarray(inputs["Wv"], np.float32) @ np.asarray(inputs["Wo"], np.float32))

    meanM = np.zeros((128, P), np.float32)
    for hp in range(128):
        meanM[hp, hp % P] = 1.0 / H
    tri = np.tile(np.tril(np.ones((P, P), np.float32), -1).reshape(1, P * P), (128, 1))
    tbase = np.tile((np.arange(SQ, dtype=np.int64) // 128 * 128)[None, :], (P, 1)).astype(np.int16)

    in_maps = []
    for c in range(NCORES):
        b, h = c // 2, c % 2
        if h == 0:
            x_ext = np.concatenate([np.zeros((1024, E), np.float32), x[b, :2048]], 0)
        else:
            x_ext = np.ascontiguousarray(x[b, 1024:4096])
        shift = np.float32(1024 - h * 2048)
        s_abs = np.arange(h * 2048, h * 2048 + SQ, dtype=np.float32)
        anchor_term = anchors[:, None] * s_abs[None, :] + shift          # [16, 2048]
        lo = (np.maximum(s_abs - MAX_DIST, 0.0) + shift).astype(np.int16)
        hi = (s_abs + shift).astype(np.int16)
        m = {
            "partition_id": np.array([[c]], np.uint32),
            "x_ext": x_ext,
            "anchor": anchor_term.astype(np.float32),
            "clip_lo": np.tile(lo[None, :], (P, 1)),
            "clip_hi": np.tile(hi[None, :], (P, 1)),
            "tbase": tbase,
            "meanM": meanM,
            "trimask": tri,
        }
        m.update(weights)
        in_maps.append(m)
    return in_maps


_CACHE = {}


def get_runner():
    """Build (once) a cached jitted SPMD callable over the 8 cores.

    Returns (run, in_names) where run takes a list of per-input np arrays
    concatenated over cores on axis 0 and returns the concatenated outputs.
    """
    if "run" in _CACHE:
        return _CACHE["run"], _CACHE["in_names"]

    import jax
    from jax.experimental.shard_map import shard_map
    from jax.sharding import Mesh, PartitionSpec
    import concourse.mybir as _mb
    from concourse.bass2jax import _bass_exec_p, install_neuronx_cc_hook

    nc = build_nc()
    install_neuronx_cc_hook()

    in_names, out_names, out_avals, zero_outs = [], [], [], []
    for alloc in nc.m.functions[0].allocations:
        if not isinstance(alloc, _mb.MemoryLocationSet):
            continue
        name = alloc.memorylocations[0].name
        if alloc.kind == "ExternalInput":
            in_names.append(name)
        elif alloc.kind == "ExternalOutput":
            out_names.append(name)
            shape = tuple(alloc.tensor_shape)
            dtype = _mb.dt.np(alloc.dtype)
            out_avals.append(jax.core.ShapedArray(shape, dtype))
            zero_outs.append(np.zeros((NCORES * shape[0], *shape[1:]), dtype))

    n_params = len(in_names)
    all_names = in_names + out_names

    def _body(*args):
        outs = _bass_exec_p.bind(
            *args,
            out_avals=tuple(out_avals),
            in_names=tuple(all_names),
            out_names=tuple(out_names),
            lowering_input_output_aliases=(),
            sim_require_finite=True,
            sim_require_nnan=True,
            nc=nc,
        )
        return tuple(outs)

    devices = jax.devices()[:NCORES]
    mesh = Mesh(np.asarray(devices), ("core",))
    sharded = jax.jit(
        shard_map(_body, mesh=mesh,
                  in_specs=(PartitionSpec("core"),) * (n_params + len(out_names)),
                  out_specs=(PartitionSpec("core"),) * len(out_names),
                  check_rep=False),
        keep_unused=True,
    )

    def run(concat_ins):
        outs = sharded(*concat_ins, *zero_outs)
        return [np.asarray(o) for o in outs]

    _CACHE.update(run=run, in_names=in_names, sharded=sharded, zero_outs=zero_outs)
    return run, in_names


def concat_inputs(in_maps, in_names):
    return [np.concatenate([np.asarray(m[n]) for m in in_maps], axis=0)
            for n in in_names]


def kernel(**inputs) -> np.ndarray:
    run, in_names = get_runner()
    in_maps = host_inputs(inputs)
    res = run(concat_inputs(in_maps, in_names))[0]   # [NCORES*SQ, E]
    out = np.zeros((B, S, E), np.float32)
    for c in range(NCORES):
        b, h = c // 2, c % 2
        out[b, h * 2048:(h + 1) * 2048] = res[c * SQ:(c + 1) * SQ]
    return out


# revision 21
# speedup vs baseline: 1.5161x; 1.0192x over previous
"""Trainium2 Bass kernel for nn_DeformableHistoryAttention_4148938408691.

Strategy (8 NeuronCores = 4 batches x 2 sequence halves, data parallel):
  Each core handles 2048 queries of one batch with a 1024-row K/V halo
  (3072 extended rows). All compute on device:
    - x transposed via PE (fp32 for the offset-MLP path, bf16 for Q/K)
    - offset MLP (W1/gelu/W2/tanh/mean) in fp32(r) -> deformable indices via
      exact round-half-even (2^23 trick), matching jnp.round bit-for-bit
      modulo fp accumulation-order noise
    - per-query index layout via 16 tiny PE transposes (no DRAM roundtrip)
    - dense windowed attention: per 128-query tile, scores over a static
      1152-wide causal window on the PE, evicted from PSUM through exp()
    - softmax-with-duplicates handled by multiplicity: an index-dedup pass
      produces per-query counts; gpsimd local_scatter writes the counts into
      the dense window; wdE = counts * exp(scores) gives unnormalized
      attention, Z = rowsum(wdE), and 1/Z is folded into the final output
      scaling (where partitions are queries)
    - AV uses associativity: attn @ (x@Wv) @ Wo == (attn @ x) @ (Wv@Wo);
      the fused weight Wf = Wv@Wo is precomputed on host, eliminating the
      V projection and the separate output projection
  Everything except the index path runs in bf16 with fp32 PSUM accumulation.
"""

import os
import sys

for _p in ("/opt/trn_rl_repo", "/root/.axon_site/_ro/trn_rl_repo"):
    if os.path.isdir(_p) and _p not in sys.path:
        sys.path.append(_p)

import dataclasses
from contextlib import ExitStack

import numpy as np

import concourse.bass as bass
import concourse.mybir as mybir
import concourse.tile as tile
from concourse import bacc
from concourse._compat import with_exitstack
from concourse.masks import make_identity
from concourse import library_config
from concourse.tile import add_dep_helper

F32 = mybir.dt.float32
F32R = mybir.dt.float32r
BF16 = mybir.dt.bfloat16
FP8 = mybir.dt.float8e4
I16 = mybir.dt.int16
FP8_SCORES = True      # Q/K in e4m3 for the dense score matmuls (2x PE rate)
FP8_PROJ = True        # Q/K projections in e4m3 DoubleRow (x^T and Wq/Wk fp8)
WSCALE = 16.0          # fp8 pre-scale for Wq/Wk (values ~0.02 -> ~0.32)
AF = mybir.ActivationFunctionType
ALU = mybir.AluOpType

E = 512            # embed dim
H = 8              # heads
P = 16             # points
MAX_DIST = 1024
OFFSET_SCALE = 8.0
B, S = 4, 4096
NCORES = 8
SQ = 2048          # queries per core
EXT = 3072         # extended rows per core (1024 halo + 2048)
NT = 16            # query tiles of 128
W = 1152           # dense window width (1024 + 128)
EC = 4             # embed chunks of 128
RC = EXT // 128    # 24 row chunks
QRC = SQ // 128    # 16 query row chunks
RNE_C = float(2.0 ** 23)
N_GENERAL = 3      # tiles using general pairwise dedup (unsorted possible)


@with_exitstack
def _emit(ctx: ExitStack, tc: tile.TileContext, io: dict, reps: int = 1):
    nc = tc.nc

    const = ctx.enter_context(tc.tile_pool(name="const", bufs=1))

    # ---- identities & small constants ----
    ident_f = const.tile([128, 128], F32)
    make_identity(nc, ident_f)
    ident_b = const.tile([128, 128], BF16)
    make_identity(nc, ident_b)

    meanMf = const.tile([128, P], F32)
    nc.gpsimd.dma_start(meanMf[:], io["meanM"][:])
    meanM = const.tile([128, P], F32R)
    nc.vector.tensor_copy(meanM[:], meanMf[:])
    anchor = const.tile([P, SQ], F32)
    nc.gpsimd.dma_start(anchor[:], io["anchor"][:])
    clip_lo = const.tile([P, SQ], I16)
    nc.gpsimd.dma_start(clip_lo[:], io["clip_lo"][:])
    clip_hi = const.tile([P, SQ], I16)
    nc.gpsimd.dma_start(clip_hi[:], io["clip_hi"][:])
    tbase = const.tile([P, SQ], I16)
    nc.gpsimd.dma_start(tbase[:], io["tbase"][:])
    trimask = const.tile([128, P * P], F32)
    nc.gpsimd.dma_start(trimask[:], io["trimask"][:])

    # ---- weights ----
    # index-path weights as fp32r (walrus requires rounded producers)
    W1f = const.tile([128, EC, E], F32)
    nc.gpsimd.dma_start(W1f[:], io["W1"][:].rearrange("(kc p) m -> p kc m", p=128))
    W1s = const.tile([128, EC, E], F32R)
    nc.vector.tensor_copy(W1s[:], W1f[:])
    W2f = const.tile([128, EC, H * P], F32)
    nc.gpsimd.dma_start(W2f[:], io["W2"][:].rearrange("(kc p) m -> p kc m", p=128))
    W2s = const.tile([128, EC, H * P], F32R)
    nc.vector.tensor_copy(W2s[:], W2f[:])
    # bf16 weight tiles (DMAs emitted after phase 1 so x loads go first)
    wqk_dt = FP8 if FP8_PROJ else BF16
    Wqs = const.tile([128, EC, E], wqk_dt)
    Wks = const.tile([128, EC, E], wqk_dt)
    Wqb = const.tile([128, EC, E], BF16)   # DMA-cast staging
    Wkb = const.tile([128, EC, E], BF16)
    Wfs = const.tile([128, EC, E], BF16)   # fused Wv @ Wo

    for _rep in range(reps):
      with tc.tile_pool(name="persist", bufs=1) as persist:
        # ---- persistent activations ----
        xT_dt = FP8 if FP8_PROJ else BF16
        xTb = persist.tile([128, EC, EXT], xT_dt)     # x^T (for Q/K projections)
        Xn = persist.tile([128, RC, E], BF16)         # x natural bf16 (for AV)
        qk_dt = FP8 if FP8_SCORES else BF16
        KT = persist.tile([128, EC, EXT], qk_dt)      # K^T
        QT = persist.tile([128, EC, SQ], qk_dt)       # Q^T
        idxS = persist.tile([128, NT, P], I16)        # per-query indices
        cnt = persist.tile([128, NT, P], F32)
        rep = persist.tile([128, NT, P], I16)
        cntv = persist.tile([128, NT, P], BF16)       # scatter values (counts)
        idxm = persist.tile([128, NT, P], I16)        # scatter indices (-1 = skip)
        rzt = persist.tile([128, NT], F32)            # 1/Z per query

        x_dram = io["x_ext"]
        out_dram = io["out"]

        # ================= Phase 1+2: x load, transpose, MLP -> indices ========
        with tc.tile_pool(name="ph12", bufs=2) as ph12, \
             tc.tile_pool(name="ph12p", bufs=2, space="PSUM") as ph12p:

            idx_f = ph12.tile([P, SQ], F32, bufs=1)

            # transpose all 24 row-chunks; fp32 x^T kept only for query rows,
            # streamed per 512-query group through the MLP
            xTf = None
            x_rows = x_dram[:].rearrange("(r p) e -> r p e", p=128)
            for rc in range(RC):
                xcht = ph12.tile([128, E], F32, tag="xch", bufs=6)
                eng = (nc.sync, nc.scalar)[rc % 2]
                eng.dma_start(xcht[:], x_rows[rc])
                xch = xcht[:]
                nc.vector.tensor_copy(Xn[:, rc, :], xch)
                is_q = rc >= 8
                qrc = rc - 8            # query row chunk 0..15
                if is_q and qrc % 4 == 0:
                    xTf = ph12.tile([128, EC, 512], F32R, tag="xTf")
                for ec in range(EC):
                    tp = ph12p.tile([128, 128], F32, tag="tp")
                    nc.tensor.transpose(tp[:], xch[:, ec * 128:(ec + 1) * 128], ident_f[:])
                    nc.scalar.activation(xTb[:, ec, rc * 128:(rc + 1) * 128], tp[:], AF.Copy)
                    if is_q:
                        nc.vector.tensor_copy(xTf[:, ec, (qrc % 4) * 128:(qrc % 4 + 1) * 128], tp[:])
                if is_q and qrc % 4 == 3:
                    sc = qrc // 4       # 512-query group
                    ssl = slice(sc * 512, (sc + 1) * 512)
                    # MLP: h^T = gelu(W1^T x^T)
                    hT = ph12.tile([128, EC, 512], F32R, tag="hT", bufs=1)
                    for e1c in range(EC):
                        hp = ph12p.tile([128, 512], F32, tag="hp")
                        for kc in range(EC):
                            nc.tensor.matmul(hp[:], W1s[:, kc, e1c * 128:(e1c + 1) * 128],
                                             xTf[:, kc, :], start=kc == 0, stop=kc == EC - 1)
                        nc.scalar.activation(hT[:, e1c, :], hp[:], AF.Gelu)
                    # offsets: tanh(W2^T h^T)
                    op = ph12p.tile([128, 512], F32, tag="op")
                    for e1c in range(EC):
                        nc.tensor.matmul(op[:], W2s[:, e1c, :], hT[:, e1c, :],
                                         start=e1c == 0, stop=e1c == EC - 1)
                    tanhT = ph12.tile([128, 512], F32R, tag="tanhT")
                    nc.scalar.activation(tanhT[:], op[:], AF.Tanh)
                    # mean over heads: [16, 512]
                    mp = ph12p.tile([P, 512], F32, tag="mp")
                    nc.tensor.matmul(mp[:], meanM[:], tanhT[:], start=True, stop=True)
                    # sampled = clip(anchor + 8*mean, lo, hi); idx = rne(sampled)
                    sf = ph12.tile([P, 512], F32, tag="sf")
                    nc.vector.scalar_tensor_tensor(sf[:], mp[:], float(OFFSET_SCALE),
                                                   anchor[:, ssl], op0=ALU.mult, op1=ALU.add)
                    nc.vector.tensor_tensor(sf[:], sf[:], clip_lo[:, ssl], op=ALU.max)
                    nc.vector.tensor_tensor(sf[:], sf[:], clip_hi[:, ssl], op=ALU.min)
                    nc.vector.tensor_scalar_add(sf[:], sf[:], RNE_C)
                    nc.vector.tensor_scalar_add(sf[:], sf[:], -RNE_C)
                    nc.vector.tensor_tensor(idx_f[:, ssl], sf[:], tbase[:, ssl], op=ALU.subtract)

            # ---- per-query index layout via tiny PE transposes ----
            idxS_f = ph12.tile([128, NT, P], F32, bufs=1, tag="idxSf")
            for t in range(NT):
                tpi = ph12p.tile([128, 128], F32, tag="tp")
                nc.tensor.transpose(tpi[:, 0:P], idx_f[:, t * 128:(t + 1) * 128],
                                    ident_f[0:P, 0:P])
                nc.vector.tensor_copy(idxS_f[:, t, :], tpi[:, 0:P])
            nc.vector.tensor_copy(idxS[:], idxS_f[:])

        # bf16 weights (cast during DMA via SWDGE)
        nc.gpsimd.dma_start(Wqb[:], io["Wq"][:].rearrange("(kc p) m -> p kc m", p=128))
        nc.gpsimd.dma_start(Wkb[:], io["Wk"][:].rearrange("(kc p) m -> p kc m", p=128))
        nc.gpsimd.dma_start(Wfs[:], io["Wf"][:].rearrange("(kc p) m -> p kc m", p=128))
        if FP8_PROJ:
            nc.vector.tensor_scalar_mul(Wqs[:], Wqb[:], WSCALE)
            nc.vector.tensor_scalar_mul(Wks[:], Wkb[:], WSCALE)
        else:
            nc.vector.tensor_copy(Wqs[:], Wqb[:])
            nc.vector.tensor_copy(Wks[:], Wkb[:])

        # ---- dedup: cnt = multiplicity, rep = first-occurrence mask ----
        nc.vector.memset(cnt[:], 1.0)
        eqt = persist.tile([128, NT, P], F32)
        for L in range(1, P):
            nc.vector.tensor_tensor(eqt[:, :, :P - L], idxS[:, :, L:], idxS[:, :, :P - L],
                                    op=ALU.is_equal)
            nc.vector.tensor_tensor(cnt[:, :, :P - L], cnt[:, :, :P - L], eqt[:, :, :P - L],
                                    op=ALU.add)
        nc.vector.memset(rep[:, :, 0:1], 1.0)
        nc.vector.tensor_tensor(rep[:, :, 1:], idxS[:, :, 1:], idxS[:, :, :P - 1],
                                op=ALU.not_equal)
        # general pairwise for the first N_GENERAL tiles (may be unsorted)
        eqm = persist.tile([128, N_GENERAL, P, P], F32)
        in0 = idxS[:, :N_GENERAL, :].to_broadcast([128, N_GENERAL, P, P])
        in1 = in0.rearrange("c t p b -> c t b p")
        nc.vector.tensor_tensor(eqm[:], in0, in1, op=ALU.is_equal)
        nc.vector.reduce_sum(cnt[:, :N_GENERAL, :], eqm[:], axis=mybir.AxisListType.X)
        tri = trimask[:].rearrange("c (p b) -> c p b", p=P)
        tri = dataclasses.replace(
            tri, ap=[tri.ap[0], [0, N_GENERAL], tri.ap[1], tri.ap[2]])
        nc.vector.tensor_tensor(eqm[:], eqm[:], tri, op=ALU.mult)
        nbef = persist.tile([128, N_GENERAL, P], F32)
        nc.vector.reduce_sum(nbef[:], eqm[:], axis=mybir.AxisListType.X)
        nc.vector.tensor_scalar(rep[:, :N_GENERAL, :], nbef[:], 0.0, None, op0=ALU.is_equal)

        nc.vector.tensor_copy(cntv[:], cnt[:])
        nc.vector.memset(idxm[:], -1)
        nc.vector.copy_predicated(idxm[:], rep[:], idxS[:])

        # ================= Phase 4: projections (Q^T, K^T) ======================
        def _proj_mm(pp, Wt, mc, src_sl):
            if FP8_PROJ:
                for kc in (0, 2):
                    nc.tensor.matmul(pp[:], Wt[:, kc:kc + 2, mc * 128:(mc + 1) * 128],
                                     xTb[:, kc:kc + 2, src_sl],
                                     start=kc == 0, stop=kc == 2,
                                     perf_mode=mybir.MatmulPerfMode.DoubleRow)
            else:
                for kc in range(EC):
                    nc.tensor.matmul(pp[:], Wt[:, kc, mc * 128:(mc + 1) * 128],
                                     xTb[:, kc, src_sl], start=kc == 0, stop=kc == EC - 1)

        with tc.tile_pool(name="psum_s", bufs=2, space="PSUM") as psum_s:
            for mc in range(EC):        # K^T / Q^T output embed chunk
                for nc_i in range(RC // 4):   # 512-col groups of ext rows
                    ksl = slice(nc_i * 512, (nc_i + 1) * 512)
                    kp = psum_s.tile([128, 512], F32, tag="projp")
                    _proj_mm(kp, Wks, mc, ksl)
                    nc.scalar.activation(KT[:, mc, ksl], kp[:], AF.Copy)
                for nc_i in range(QRC // 4):
                    qsl = slice(1024 + nc_i * 512, 1024 + (nc_i + 1) * 512)
                    qp = psum_s.tile([128, 512], F32, tag="projp")
                    _proj_mm(qp, Wqs, mc, qsl)
                    nc.scalar.activation(QT[:, mc, slice(nc_i * 512, (nc_i + 1) * 512)], qp[:], AF.Copy)

        # ================= Phase 5: scores+exp, scatter, weights, AV ============
        lib7 = nc.gpsimd.load_library(library_config.local_scatter)
        NCHUNKS = ((0, 512), (512, 512), (1024, 128))
        NP_PAIR = NT // 2
        with tc.tile_pool(name="ph5", bufs=2) as ph5, \
             tc.tile_pool(name="ph5s", bufs=2, space="PSUM") as ph5s, \
             tc.tile_pool(name="ph5t", bufs=2, space="PSUM") as ph5t, \
             tc.tile_pool(name="ph5a", bufs=1, space="PSUM") as ph5a, \
             tc.tile_pool(name="ph5o", bufs=2, space="PSUM") as ph5o:
            for pr in range(NP_PAIR):
                wT = ph5.tile([128, 10, 256], BF16, tag="wT")
                nc.vector.memset(wT[:, 9, 0:128], 0.0)
                nc.vector.memset(wT[:, 0, 128:256], 0.0)
                wdE_pair = []
                for wh in range(2):
                    t = pr * 2 + wh
                    # dense scores, evicted from PSUM through exp()
                    Et = ph5.tile([128, W], BF16, tag="Et")
                    for noff, nw in NCHUNKS:
                        sp = ph5s.tile([128, 512], F32, tag="sp")
                        tsl = slice(t * 128, (t + 1) * 128)
                        wsl = slice(t * 128 + noff, t * 128 + noff + nw)
                        if FP8_SCORES:
                            # DoubleRow: two ec-chunk contractions per pass
                            for pe in (0, 2):
                                nc.tensor.matmul(
                                    sp[:, 0:nw], QT[:, pe:pe + 2, tsl],
                                    KT[:, pe:pe + 2, wsl],
                                    start=pe == 0, stop=pe == 2,
                                    perf_mode=mybir.MatmulPerfMode.DoubleRow)
                        else:
                            for ec in range(EC):
                                nc.tensor.matmul(sp[:, 0:nw],
                                                 QT[:, ec, tsl], KT[:, ec, wsl],
                                                 start=ec == 0, stop=ec == EC - 1)
                        s_scale = 1.0 / np.sqrt(E)
                        if FP8_PROJ:
                            s_scale /= WSCALE * WSCALE
                        nc.scalar.activation(Et[:, noff:noff + nw], sp[:, 0:nw], AF.Exp,
                                             scale=float(s_scale))
                    # multiplicity counts scattered into the dense window
                    wd = ph5.tile([128, W], BF16, tag="wd")
                    si = nc.gpsimd.local_scatter(wd[:], cntv[:, t, :], idxm[:, t, :],
                                                 channels=128, num_elems=W, num_idxs=P)
                    add_dep_helper(si.ins, lib7.ins, False, "lib7 before scatters")
                    # unnormalized attention + row sum
                    wdE = ph5.tile([128, W], BF16, tag="wdE")
                    nc.vector.tensor_tensor(wdE[:], wd[:], Et[:], op=ALU.mult)
                    zs = ph5.tile([128, 1], F32, tag="zs")
                    nc.vector.reduce_sum(zs[:], wdE[:], axis=mybir.AxisListType.X)
                    nc.vector.reciprocal(rzt[:, t:t + 1], zs[:])
                    wdE_pair.append(wdE)
                # transposes after both tiles' scores: while the PE works on
                # tile t1's scores, t0's scatter/mult complete off-engine
                for wh in range(2):
                    wdE = wdE_pair[wh]
                    for jc in range(9):
                        tpb = ph5t.tile([128, 128], BF16, tag="tpb")
                        nc.tensor.transpose(tpb[:], wdE[:, jc * 128:(jc + 1) * 128], ident_b[:])
                        nc.vector.tensor_copy(wT[:, jc + wh, wh * 128:(wh + 1) * 128], tpb[:])
                avp = ph5a.tile([128, EC * 256], F32, tag="avp")
                for ec in range(EC):
                    for jc in range(10):
                        nc.tensor.matmul(avp[:, ec * 256:(ec + 1) * 256],
                                         Xn[:, pr * 2 + jc, ec * 128:(ec + 1) * 128],
                                         wT[:, jc, :], start=jc == 0, stop=jc == 9)
                avT = ph5.tile([128, EC, 256], BF16, tag="avT")
                nc.vector.tensor_copy(avT[:], avp[:].rearrange("c (e s) -> c e s", e=EC))
                for wh in range(2):
                    t = pr * 2 + wh
                    wop = ph5o.tile([128, E], F32, tag="wop")
                    for ec in range(EC):
                        nc.tensor.matmul(wop[:], avT[:, ec, wh * 128:(wh + 1) * 128],
                                         Wfs[:, ec, :], start=ec == 0, stop=ec == EC - 1)
                    osb = ph5.tile([128, E], F32, tag="osb")
                    nc.scalar.activation(osb[:], wop[:], AF.Copy,
                                         scale=rzt[:, t:t + 1])
                    nc.sync.dma_start(
                        out_dram[:].rearrange("(t p) e -> t p e", p=128)[t], osb[:])


def build_nc(reps: int = 1):
    nc = bacc.Bacc("TRN2", target_bir_lowering=False, debug=False)
    io = {}
    io["x_ext"] = nc.declare_dram_parameter("x_ext", [EXT, E], F32, isOutput=False).ap()
    for nm in ("Wq", "Wk", "Wf", "W1", "W2"):
        shp = [E, H * P] if nm == "W2" else [E, E]
        io[nm] = nc.declare_dram_parameter(nm, shp, F32, isOutput=False).ap()
    io["anchor"] = nc.declare_dram_parameter("anchor", [P, SQ], F32, isOutput=False).ap()
    io["clip_lo"] = nc.declare_dram_parameter("clip_lo", [P, SQ], I16, isOutput=False).ap()
    io["clip_hi"] = nc.declare_dram_parameter("clip_hi", [P, SQ], I16, isOutput=False).ap()
    io["tbase"] = nc.declare_dram_parameter("tbase", [P, SQ], I16, isOutput=False).ap()
    io["meanM"] = nc.declare_dram_parameter("meanM", [128, P], F32, isOutput=False).ap()
    io["trimask"] = nc.declare_dram_parameter("trimask", [128, P * P], F32, isOutput=False).ap()
    io["out"] = nc.declare_dram_parameter("out", [SQ, E], F32, isOutput=True).ap()

    with tile.TileContext(nc) as tc:
        _emit(tc, io, reps=reps)
    nc.finalize()
    return nc


def host_inputs(inputs: dict) -> list:
    """Build the 8 per-core input maps from the full problem inputs."""
    x = np.asarray(inputs["x"], np.float32)
    anchors = np.asarray(inputs["anchors"], np.float32)
    weights = {k: np.ascontiguousarray(np.asarray(inputs[k], np.float32))
               for k in ("Wq", "Wk", "W1", "W2")}
    weights["Wf"] = np.ascontiguousarray(
        np.asarray(inputs["Wv"], np.float32) @ np.asarray(inputs["Wo"], np.float32))

    meanM = np.zeros((128, P), np.float32)
    for hp in range(128):
        meanM[hp, hp % P] = 1.0 / H
    tri = np.tile(np.tril(np.ones((P, P), np.float32), -1).reshape(1, P * P), (128, 1))
    tbase = np.tile((np.arange(SQ, dtype=np.int64) // 128 * 128)[None, :], (P, 1)).astype(np.int16)

    in_maps = []
    for c in range(NCORES):
        b, h = c // 2, c % 2
        if h == 0:
            x_ext = np.concatenate([np.zeros((1024, E), np.float32), x[b, :2048]], 0)
        else:
            x_ext = np.ascontiguousarray(x[b, 1024:4096])
        shift = np.float32(1024 - h * 2048)
        s_abs = np.arange(h * 2048, h * 2048 + SQ, dtype=np.float32)
        anchor_term = anchors[:, None] * s_abs[None, :] + shift          # [16, 2048]
        lo = (np.maximum(s_abs - MAX_DIST, 0.0) + shift).astype(np.int16)
        hi = (s_abs + shift).astype(np.int16)
        m = {
            "partition_id": np.array([[c]], np.uint32),
            "x_ext": x_ext,
            "anchor": anchor_term.astype(np.float32),
            "clip_lo": np.tile(lo[None, :], (P, 1)),
            "clip_hi": np.tile(hi[None, :], (P, 1)),
            "tbase": tbase,
            "meanM": meanM,
            "trimask": tri,
        }
        m.update(weights)
        in_maps.append(m)
    return in_maps


_CACHE = {}


def get_runner():
    """Build (once) a cached jitted SPMD callable over the 8 cores.

    Returns (run, in_names) where run takes a list of per-input np arrays
    concatenated over cores on axis 0 and returns the concatenated outputs.
    """
    if "run" in _CACHE:
        return _CACHE["run"], _CACHE["in_names"]

    import jax
    from jax.experimental.shard_map import shard_map
    from jax.sharding import Mesh, PartitionSpec
    import concourse.mybir as _mb
    from concourse.bass2jax import _bass_exec_p, install_neuronx_cc_hook

    nc = build_nc()
    install_neuronx_cc_hook()

    in_names, out_names, out_avals, zero_outs = [], [], [], []
    for alloc in nc.m.functions[0].allocations:
        if not isinstance(alloc, _mb.MemoryLocationSet):
            continue
        name = alloc.memorylocations[0].name
        if alloc.kind == "ExternalInput":
            in_names.append(name)
        elif alloc.kind == "ExternalOutput":
            out_names.append(name)
            shape = tuple(alloc.tensor_shape)
            dtype = _mb.dt.np(alloc.dtype)
            out_avals.append(jax.core.ShapedArray(shape, dtype))
            zero_outs.append(np.zeros((NCORES * shape[0], *shape[1:]), dtype))

    n_params = len(in_names)
    all_names = in_names + out_names

    def _body(*args):
        outs = _bass_exec_p.bind(
            *args,
            out_avals=tuple(out_avals),
            in_names=tuple(all_names),
            out_names=tuple(out_names),
            lowering_input_output_aliases=(),
            sim_require_finite=True,
            sim_require_nnan=True,
            nc=nc,
        )
        return tuple(outs)

    devices = jax.devices()[:NCORES]
    mesh = Mesh(np.asarray(devices), ("core",))
    sharded = jax.jit(
        shard_map(_body, mesh=mesh,
                  in_specs=(PartitionSpec("core"),) * (n_params + len(out_names)),
                  out_specs=(PartitionSpec("core"),) * len(out_names),
                  check_rep=False),
        keep_unused=True,
    )

    def run(concat_ins):
        outs = sharded(*concat_ins, *zero_outs)
        return [np.asarray(o) for o in outs]

    _CACHE.update(run=run, in_names=in_names, sharded=sharded, zero_outs=zero_outs)
    return run, in_names


def concat_inputs(in_maps, in_names):
    return [np.concatenate([np.asarray(m[n]) for m in in_maps], axis=0)
            for n in in_names]


def kernel(**inputs) -> np.ndarray:
    run, in_names = get_runner()
    in_maps = host_inputs(inputs)
    res = run(concat_inputs(in_maps, in_names))[0]   # [NCORES*SQ, E]
    out = np.zeros((B, S, E), np.float32)
    for c in range(NCORES):
        b, h = c // 2, c % 2
        out[b, h * 2048:(h + 1) * 2048] = res[c * SQ:(c + 1) * SQ]
    return out
